# revision 2
# baseline (speedup 1.0000x reference)
"""GAT layer kernel v2 for Trainium2 (Bass/Tile), SPMD over 8 NeuronCores.

Problem (fixed shapes, fp32):
    x: [8, 2048, 128], W: [4, 128, 64], b: [4, 64], a: [4, 128]
    h    = x @ W + b                    (per head)          [B,H,N,64]
    e    = leaky_relu(f_i[:,None] + f_j[None,:], 0.2)       [B,H,N,N]
    attn = softmax(e, axis=-1)
    out  = mean_h(attn @ h)                                 [B,N,64]
  where f_i = h @ a1, f_j = h @ a2.

Sharding: data-parallel, one batch element per core (B == 8 == n_cores).

Math (same reformulation as v1):
  exp(leaky(s)) = max(exp(s), exp(0.2 s)); softmax rows are scale-invariant,
  so with c = f_i, g = f_j:
      Z[j,i] = max(exp(0.8 c_i) * exp(g_j), exp(0.2 g_j))
      out[i,:] = (sum_j Z[j,i] h[j,:]) / (sum_j Z[j,i]).
  oT[o,i] += [h | 4.0][j,o].T @ Z[j,i] accumulated over j tiles on the PE;
  row 64 of oT is 4*denominator (4 bakes in the head mean).

v2 structural changes (all measured on HW via microbenchmarks):
  * x is transposed on the HOST (free) -> no on-device transposes of x,
    no f32->f32r copies: xT/W/bias dram tensors are float32r directly.
  * Everything fp16 on the z/attn path (fp8 DoubleRow measured ~= fp16
    per unit work on this HW; GPSIMD tensor_scalar measured 30 us/tile).
  * Software-pipelined hw_loop body: [A(cur); S(next)] x2 unrolled with
    explicit A/B ping-pong buffers. Setup PE work (c-rows, h-gen) is
    interleaved into the attention matmul stream in "slots"; ACT/DVE/Pool
    streams follow emission order per engine.
  * Normalize: PE transposes raw oT tiles (4-tile groups into one PSUM
    bank), DVE grouped reciprocal of the 4 denominator columns, ACT
    Copy-with-scale(rec) per (h,it) -> oTn[h] fp16, DVE pairwise adds for
    the head sum -> acc f32, 4 output DMAs on the ACT queue.
  * Pool does only the 4 partition_broadcasts (7.7 us each on HW),
    prefetched an iteration ahead, off the critical path.
"""

import os
import sys

import numpy as np

_TRN_REPO = "/opt/trn_rl_repo"
if _TRN_REPO not in sys.path and os.path.isdir(_TRN_REPO):
    sys.path.insert(0, _TRN_REPO)

B, N, IN, OUT, H = 8, 2048, 128, 64, 4
NCORES = 8
P = 128
T = N // P            # 16 node tiles
IBS = 512             # i-block (PSUM bank) size
NIB = N // IBS        # 4
HO = OUT + 1          # 65: [h | ones] block width
CB = H * HO           # 260: per-tile width of the fused hw block
GB = CB               # g cols start right after hw cols in the h-gen psum
WC2 = CB + 2 * H      # 268: h-gen psum width (hw | g | 0.2*g)


def _build_program(hw_loop=0, z_bufs=6, oT_bufs=4, norm_act=True, debug_dump=False,
                   abl_no_norm=False, abl_no_s=False, abl_no_drain=False):
    import concourse.bass as bass
    import concourse.tile as tile
    from concourse import bacc, mybir

    f32 = mybir.dt.float32
    f32r = mybir.dt.float32r
    f16 = mybir.dt.float16
    Exp = mybir.ActivationFunctionType.Exp
    Copy = mybir.ActivationFunctionType.Copy
    mult = mybir.AluOpType.mult
    amax = mybir.AluOpType.max
    add = mybir.AluOpType.add

    nc = bacc.Bacc("TRN2", target_bir_lowering=False, debug=False)

    xt_d = nc.dram_tensor("xt", [IN, N], f32r, kind="ExternalInput")
    wf_d = nc.dram_tensor("wf", [IN, WC2], f32r, kind="ExternalInput")
    bias_d = nc.dram_tensor("bias", [1, WC2], f32r, kind="ExternalInput")
    wfc_d = nc.dram_tensor("wfc", [IN, H], f32r, kind="ExternalInput")
    cb08c_d = nc.dram_tensor("cb08c", [H, 1], f32, kind="ExternalInput")
    ones_d = nc.dram_tensor("ones", [1, P], f32r, kind="ExternalInput")
    ident_d = nc.dram_tensor("ident", [P, P], f32, kind="ExternalInput")
    out_d = nc.dram_tensor("out", [N, OUT], f32, kind="ExternalOutput")
    e08s_d = nc.dram_tensor("e08scratch", [H, N], mybir.dt.float16)
    if debug_dump:
        dbg_hw = nc.dram_tensor("dbg_hw", [P, T * CB], mybir.dt.float16, kind="ExternalOutput")
        dbg_u = nc.dram_tensor("dbg_u", [P, T * 2 * H], f32, kind="ExternalOutput")
        dbg_e08 = nc.dram_tensor("dbg_e08", [H, N], mybir.dt.float16, kind="ExternalOutput")
        dbg_v08 = nc.dram_tensor("dbg_v08", [H, P, N], mybir.dt.float16, kind="ExternalOutput")
        dbg_oTn = nc.dram_tensor("dbg_oTn", [H, P, T * OUT], mybir.dt.float16, kind="ExternalOutput")

    with tile.TileContext(nc) as tc:
        with tc.tile_pool(name="const", bufs=1) as cpool:
            wf_sb = cpool.tile([IN, WC2], f32r, tag="wf")
            nc.sync.dma_start(wf_sb[:], wf_d.ap())
            bias_sb = cpool.tile([1, WC2], f32r, tag="bias")
            nc.sync.dma_start(bias_sb[:], bias_d.ap())
            wfc_sb = cpool.tile([IN, H], f32r, tag="wfc")
            nc.sync.dma_start(wfc_sb[:], wfc_d.ap())
            cb08c_sb = cpool.tile([H, 1], f32, tag="cb08c")
            nc.sync.dma_start(cb08c_sb[:], cb08c_d.ap())
            ones_sb = cpool.tile([1, P], f32r, tag="ones")
            nc.sync.dma_start(ones_sb[:], ones_d.ap())
            ident_sb = cpool.tile([P, P], f32, tag="ident")
            nc.sync.dma_start(ident_sb[:], ident_d.ap())

            # ping/pong S-phase buffers (explicit, hw_loop-safe)
            xT = [cpool.tile([IN, N], f32r, tag=f"xT{i}", name=f"xT{i}") for i in range(2)]
            hw16 = [cpool.tile([P, T * CB], f16, tag=f"hw{i}", name=f"hw16_{i}") for i in range(2)]
            # u12 layout: [128, T*8]; cols t*8+h = exp(g), t*8+4+h = exp(0.2 g)
            u12 = [cpool.tile([P, T * 2 * H], f32, tag=f"u{i}", name=f"u12_{i}") for i in range(2)]
            # fused c-rows exp(0.8 c): [4, N], partition h = head h
            e08 = [cpool.tile([H, N], f16, tag=f"e08_{i}", name=f"e08_{i}") for i in range(2)]
            # v08bc: one [128, N] fp16 tile per head, single set (overwritten
            # by the next iteration's broadcast after all readers are done)
            v08 = [cpool.tile([P, N], f16, tag=f"v08_{h}", name=f"v08_{h}") for h in range(H)]
            # per-head normalized fp16 outputs [i, o] layout
            oTn = [cpool.tile([P, T * OUT], f16, tag=f"oTn_{h}", name=f"oTn_{h}") for h in range(H)]
            t01 = cpool.tile([P, T * OUT], f16, tag="t01")
            t23 = cpool.tile([P, T * OUT], f16, tag="t23")

            def dma_x(nxt):
                # two halves on the sync (SP) queue
                for k in range(2):
                    nc.sync.dma_start(
                        xT[nxt][:, k * (N // 2) : (k + 1) * (N // 2)],
                        xt_d.ap()[:, k * (N // 2) : (k + 1) * (N // 2)],
                    )

            def emit_crow(spool, nxt, ib, tag):
                # c-rows for ALL 4 heads at once: [4, 512] = wfc.T @ xT-slice
                psc = spool.tile([H, IBS], f32, tag="crow", name=f"crow{tag}_{ib}")
                nc.tensor.matmul(
                    psc[:],
                    wfc_sb[:],
                    xT[nxt][:, ib * IBS : (ib + 1) * IBS],
                    start=True,
                    stop=True,
                )
                nc.scalar.activation(
                    e08[nxt][0:H, ib * IBS : (ib + 1) * IBS],
                    psc[:],
                    Exp,
                    scale=0.8,
                    bias=cb08c_sb[0:H, 0:1],
                )
                if ib == NIB - 1:
                    # bounce the whole [4, N] c-row block to DRAM (scalar q)
                    nc.scalar.dma_start(e08s_d.ap(), e08[nxt][:])

            def emit_hgen(spool, nxt, t, tag):
                ps = spool.tile([P, WC2], f32, tag="hgen", name=f"hgen{tag}_{t}")
                nc.tensor.matmul(ps[:], ones_sb[:], bias_sb[:], start=True, stop=False)
                nc.tensor.matmul(
                    ps[:],
                    xT[nxt][:, t * P : (t + 1) * P],
                    wf_sb[:],
                    start=False,
                    stop=True,
                )
                # one exp for u1 (g cols) and u2 (0.2g cols) together
                nc.scalar.activation(
                    u12[nxt][:, t * 2 * H : (t + 1) * 2 * H],
                    ps[:, GB : GB + 2 * H],
                    Exp,
                    scale=1.0,
                )
                nc.scalar.copy(
                    hw16[nxt][:, t * CB : (t + 1) * CB], ps[:, 0:CB]
                )

            def emit_bcast(nxt, h):
                sl = e08s_d.ap()[h : h + 1, :]
                bcast_ap = bass.AP(
                    tensor=sl.tensor, offset=sl.offset, ap=[[0, P]] + sl.ap[1:]
                )
                nc.scalar.dma_start(v08[h][:], bcast_ap)

            def emit_iter(pools, cur, nxt, tag):
                """A-phase for buffers `cur`, S-phase into `nxt` (None: skip)."""
                (zpool, oTp, trp, spool, oTsb, smallp, accp) = pools
                do_s = (nxt is not None) and not abl_no_s
                if do_s:
                    dma_x(nxt)

                # PE slot work-queues: list of callables
                slots = []
                if do_s:
                    for ib in range(NIB):
                        slots.append(lambda ib=ib: emit_crow(spool, nxt, ib, tag))
                    for t in range(T):
                        slots.append(lambda t=t: emit_hgen(spool, nxt, t, tag))
                si = 0

                # deferred normalize chunks: one per completed head
                norm_q = []

                stage2_q = []

                def emit_norm_stage2():
                    if not stage2_q:
                        return False
                    h, g, pst = stage2_q.pop(0)
                    rec4 = smallp.tile([P, 4], f32, tag="rec4")
                    den_ap = bass.AP(
                        tensor=pst[:].tensor,
                        offset=pst[:, OUT : OUT + 1].offset,
                        ap=[pst[:].ap[0], [HO, 4]],
                    )
                    nc.vector.reciprocal(rec4[:], den_ap)
                    for k in range(4):
                        it = g * 4 + k
                        dst = oTn[h][:, it * OUT : (it + 1) * OUT]
                        src = pst[:, k * HO : k * HO + OUT]
                        if norm_act:
                            nc.scalar.activation(
                                dst, src, Copy, scale=rec4[:, k : k + 1], bias=0.0
                            )
                        else:
                            nc.vector.tensor_scalar(
                                dst, src, rec4[:, k : k + 1], None, op0=mult
                            )
                    return True

                pair_ctr = [0]

                def emit_norm_chunk():
                    # stage1: PE transposes of one it-group of a pending head;
                    # the DVE/ACT stage2 is deferred one slot so the DVE
                    # stream doesn't stall waiting on these transposes
                    if abl_no_norm:
                        norm_q.clear()
                        return False
                    if not norm_q:
                        return False
                    # delay each chunk until ~2 pairs after its head's drain
                    # so the transposes never wait on the ACT drain copies
                    if norm_q[0][3] > pair_ctr[0]:
                        return False
                    h, g, oT_sb_h, _ = norm_q.pop(0)
                    pst = trp.tile([P, 4 * HO], f32, tag="trp", name=f"trp{tag}_{h}_{g}")
                    for k in range(4):
                        it = g * 4 + k
                        nc.tensor.transpose(
                            pst[:, k * HO : (k + 1) * HO],
                            oT_sb_h[:, it * P : (it + 1) * P],
                            ident_sb[0:HO, 0:HO],
                        )
                    stage2_q.append((h, g, pst))
                    return True

                for h in range(H):
                    oT = [
                        oTp.tile([HO, IBS], f32, tag="oT", name=f"oT{tag}_{h}_{ib}")
                        for ib in range(NIB)
                    ]
                    for p in range(2 * NIB):
                        pair_ctr[0] += 1
                        for jt in (2 * p, 2 * p + 1):
                            z = zpool.tile([P, N], f16, tag="z")
                            nc.vector.tensor_scalar(
                                z[:],
                                v08[h][:],
                                u12[cur][:, jt * 2 * H + h : jt * 2 * H + h + 1],
                                u12[cur][:, jt * 2 * H + H + h : jt * 2 * H + H + h + 1],
                                op0=mult,
                                op1=amax,
                            )
                            lhs = hw16[cur][:, jt * CB + h * HO : jt * CB + (h + 1) * HO]
                            for ib in range(NIB):
                                nc.tensor.matmul(
                                    oT[ib][:],
                                    lhs,
                                    z[:, ib * IBS : (ib + 1) * IBS],
                                    start=(jt == 0),
                                    stop=(jt == T - 1),
                                )
                        # interleave: deferred normalize stage2, then a new
                        # normalize transpose chunk or one S work item
                        emit_norm_stage2()
                        if not emit_norm_chunk() and si < len(slots):
                            slots[si]()
                            si += 1
                    # drain this head's PSUM to SBUF
                    oT_sb_h = oTsb.tile([HO, N], f32, tag="oTsb", name=f"oTsb{tag}_{h}")
                    if not abl_no_drain:
                        for ib in range(NIB):
                            nc.scalar.copy(
                                oT_sb_h[:, ib * IBS : (ib + 1) * IBS], oT[ib][:]
                            )
                    for g in range(4):
                        norm_q.append((h, g, oT_sb_h, pair_ctr[0] + 2 + g))
                    # pool stream: next iteration's broadcast for head h
                    # (after head h's z instructions have all been emitted)
                    if do_s:
                        emit_bcast(nxt, h)

                # drain remaining S work and normalize chunks
                while si < len(slots):
                    slots[si]()
                    si += 1
                pair_ctr[0] += 10_000  # tail: everything is ready
                while emit_norm_chunk() or stage2_q:
                    emit_norm_stage2()

                if abl_no_norm:
                    return
                # head sum: acc = (oTn0 + oTn1) + (oTn2 + oTn3), f32
                nc.vector.tensor_tensor(t01[:], oTn[0][:], oTn[1][:], op=add)
                nc.vector.tensor_tensor(t23[:], oTn[2][:], oTn[3][:], op=add)
                acc = accp.tile([P, T * OUT], f32, tag="acc", name=f"acc{tag}")
                nc.vector.tensor_tensor(acc[:], t01[:], t23[:], op=add)

                # stores on the ACT queue (4 groups of 4 node tiles);
                # DRAM side iterates [p, tile-in-group, o] to match the
                # acc column layout acc[p, it*OUT + o] -> out[it*P + p, o]
                for g in range(4):
                    sl = out_d.ap()[g * 4 * P : (g + 1) * 4 * P, :]
                    dram_ap = bass.AP(
                        tensor=sl.tensor,
                        offset=sl.offset,
                        ap=[[OUT, P], [P * OUT, 4], [1, OUT]],
                    )
                    nc.scalar.dma_start(
                        dram_ap,
                        acc[:, g * 4 * OUT : (g + 1) * 4 * OUT],
                    )

            with (
                tc.tile_pool(name="z", bufs=z_bufs) as zpool,
                tc.tile_pool(name="oT_ps", bufs=oT_bufs, space="PSUM") as oTp,
                tc.tile_pool(name="trp_ps", bufs=2, space="PSUM") as trp,
                tc.tile_pool(name="setup_ps", bufs=1, space="PSUM") as spool,
                tc.tile_pool(name="oTsb", bufs=2) as oTsb,
                tc.tile_pool(name="small", bufs=8) as smallp,
                tc.tile_pool(name="accp", bufs=2) as accp,
            ):
                pools = (zpool, oTp, trp, spool, oTsb, smallp, accp)

                # prologue: fill buffer A
                dma_x(0)
                for ib in range(NIB):
                    emit_crow(spool, 0, ib, "P")
                for t in range(T):
                    emit_hgen(spool, 0, t, "P")
                for h in range(H):
                    emit_bcast(0, h)

                if hw_loop:
                    with tc.For_i(0, hw_loop, 1):
                        emit_iter(pools, 0, 1, "A")
                        emit_iter(pools, 1, 0, "B")
                else:
                    emit_iter(pools, 0, None, "A")
                    if debug_dump:
                        nc.sync.dma_start(dbg_hw.ap(), hw16[0][:])
                        nc.sync.dma_start(dbg_u.ap(), u12[0][:])
                        nc.sync.dma_start(dbg_e08.ap(), e08[0][:])
                        for h in range(H):
                            nc.sync.dma_start(dbg_v08.ap()[h], v08[h][:])
                            nc.sync.dma_start(dbg_oTn.ap()[h], oTn[h][:])

    nc.compile()
    return nc


def _prep_params(W, b, a):
    W = np.asarray(W, np.float32)
    b = np.asarray(b, np.float32)
    a = np.asarray(a, np.float32)
    a1, a2 = a[:, :OUT], a[:, OUT:]
    wf = np.zeros((IN, WC2), np.float32)
    bias = np.zeros((1, WC2), np.float32)
    wfc = np.zeros((IN, H), np.float32)
    cb08 = np.zeros((H, 1), np.float32)
    for h in range(H):
        wf[:, h * HO : h * HO + OUT] = W[h]
        bias[0, h * HO : h * HO + OUT] = b[h]
        bias[0, h * HO + OUT] = float(H)  # denominator scale -> head mean
        wf[:, GB + h] = W[h] @ a2[h]
        bias[0, GB + h] = float(b[h] @ a2[h])
        wf[:, GB + H + h] = 0.2 * (W[h] @ a2[h])
        bias[0, GB + H + h] = 0.2 * float(b[h] @ a2[h])
        wfc[:, h] = W[h] @ a1[h]
        cb08[h, 0] = 0.8 * float(b[h] @ a1[h])
    return wf, bias, wfc, cb08


def _make_in_maps(x, W, b, a):
    wf, bias, wfc, cb08 = _prep_params(W, b, a)
    ones = np.ones((1, P), np.float32)
    ident = np.eye(P, dtype=np.float32)
    return [
        {
            "xt": np.ascontiguousarray(x[i].T),
            "wf": wf,
            "bias": bias,
            "wfc": wfc,
            "cb08c": cb08,
            "ones": ones,
            "ident": ident,
        }
        for i in range(NCORES)
    ]


_PROGRAM = None


def kernel(x, W, b, a):
    global _PROGRAM
    from concourse import bass_utils

    x = np.asarray(x, np.float32)
    assert x.shape == (B, N, IN), x.shape

    if _PROGRAM is None:
        _PROGRAM = _build_program()
    nc = _PROGRAM

    in_maps = _make_in_maps(x, W, b, a)
    res = bass_utils.run_bass_kernel_spmd(nc, in_maps, core_ids=list(range(NCORES)))
    out = np.stack([res.results[i]["out"] for i in range(NCORES)], axis=0)
    return out.astype(np.float32)
